# revision 1
# baseline (speedup 1.0000x reference)
"""Trainium2 Bass kernel for nn_CustomLinear (block-sparse QKV projection).

Given x (8, 4096, 130), per-head 64x64 blocks M_q/M_k (4,64,64), M_v
(8,64,64) and scalar biases B_q/B_k (8,1,1), produces q, k, v each of shape
(8, 4096, 1040) = (B, N, H*E).  Per token row of 1040 floats, only a few
column blocks are nonzero:

  q: head h<4 : cols 130h+65..128  = M_q[h] @ x2,   col 130h+129 = s_last*bq[h]
     head h>=4: col  130h+65       = s_last*bq[h]
  k: head h<4 : cols 130h+65..128  = M_k[h] @ x1,   col 130h+129 = s_last*bk[h]
     head h>=4: col  130h+65       = s_mid*bk[h]
  v: all heads: cols 130h+65..128  = M_v[h] @ x1
  (x1 = x cols 0:64, x2 = x cols 65:129, s_mid = x col 64, s_last = x col 129)

Sharding: pure data parallelism, one batch row per NeuronCore (8 cores),
the tiny weights replicated.

Device kernel (per core, per 128-token tile): the bias scalars are folded
into the matmuls by extending the contraction dim with the s_mid/s_last rows
of x, so the tile is just 3 fp32 matmuls (x-tile stationary, packed weights
moving), 5 strided PSUM->SBUF copies into persistent (128, 4160) staging
buffers whose zero columns are memset once at startup, then 3 contiguous
2.1 MB DMA stores per 512-token macro tile.  The kernel is bound by the
~51 MB of f32 output DMA per core (~140 us at ~360 GB/s HBM write BW).

Host side only reshapes/transposes inputs, packs the weight matrix, and
stacks the 8 per-core outputs back to (8, 4096, 1040).
"""

import numpy as np
from contextlib import ExitStack

import concourse.bass as bass
import concourse.bacc as bacc
import concourse.mybir as mybir
import concourse.tile as tile
from concourse.bass_utils import run_bass_kernel_spmd

F32 = mybir.dt.float32
F16 = mybir.dt.float16

B = 8            # batches == cores
N = 4096         # tokens per core
D = 64
H = 8            # heads
P = 4            # pair heads
E = 130
HE = H * E       # 1040
KC = 66          # contraction rows: 64 data rows + 2 scalar rows
SUB = 128        # tokens per matmul
NSETS = 5        # stage-buffer sets per output (pipeline depth)
INTOK = 512      # tokens per input DMA tile
BUF_COLS = 2 * HE             # staging cols actually stored (2 sub-tiles)
BUF_PAD = BUF_COLS + 2 * E    # slack so rearrange slice bounds stay legal
# Macro schedule (tok0, nsub): two 128-token macros first so the output DMA
# stream starts early, then 256-token macros for full-rate 1.06 MB DMAs.
SCHED = [(0, 1), (SUB, 1)] + [(t, 2) for t in range(2 * SUB, N, 2 * SUB)]

_CACHE = {}


def _build():
    # Bacc (not raw Bass): its compile() legalizes the TRN2 one-sync-wait-
    # per-instruction constraint (move_matmul_waits_to_ldweights +
    # generate_event_semaphores), which walrus codegen hard-requires.
    nc = bacc.Bacc("TRN2", target_bir_lowering=False, debug=False)
    # fp16 high/low split of x and of the packed weight matrix: the kernel
    # computes x@W as xh@Wh + xh@Wl + xl@Wh (3 accumulating fp16 matmuls,
    # dropped xl@Wl term is ~2^-22 relative).  fp16 matmul is single-pass at
    # full PE rate; fp32 matmul is two LOW/HIGH passes at ~1/6 the rate and
    # was the critical path (218 us of PE for a ~143 us DMA roofline).
    # xp packs [xa_h, xa_l, xb_h, xb_l] so each input round is one DMA;
    # wp packs [w_h | w_l] along the free dim.
    xp = nc.dram_tensor("xp", [4, KC, N], F16, kind="ExternalInput").ap()
    wp = nc.dram_tensor("wp", [KC, 2 * HE], F16, kind="ExternalInput").ap()
    outs = {
        nm: nc.dram_tensor(nm, [N, HE], F32, kind="ExternalOutput").ap()
        for nm in ("q", "k", "v")
    }

    with tile.TileContext(nc) as tc, ExitStack() as ctx:
        wpool = ctx.enter_context(tc.tile_pool(name="wpool", bufs=1))
        xpool = ctx.enter_context(tc.tile_pool(name="xpool", bufs=2))
        opool = ctx.enter_context(tc.tile_pool(name="opool", bufs=1))
        pspool = ctx.enter_context(tc.tile_pool(name="pspool", bufs=2, space="PSUM"))

        wsb = wpool.tile([KC, 2 * HE], F16, name="wsb")
        nc.sync.dma_start(wsb[:], wp[:])
        L = HE  # offset of the low-half weights within wsb
        w_parts = {  # (high, low) weight slices per output
            "k": (wsb[:, 0:264], wsb[:, L:L + 264]),
            "v": (wsb[:, 264:776], wsb[:, L + 264:L + 776]),
            "q": (wsb[:, 776:1040], wsb[:, L + 776:L + 1040]),
        }

        stage = {
            nm: [
                opool.tile([SUB, BUF_PAD], F32, tag=f"st_{nm}{i}", name=f"st_{nm}{i}")
                for i in range(NSETS)
            ]
            for nm in ("q", "k", "v")
        }

        # Zero the statically-zero output columns of a stage buffer; they are
        # never rewritten, so every later DMA of the buffer carries them
        # along.  Emitted lazily (right before a set's first use) so the
        # first macro's output DMA isn't gated on all NSETS memsets.
        def _memset_zero_cols(nm, t):
            # on gpsimd: the DVE is busy with PSUM->stage copies during the
            # pipeline ramp, and these memsets would starve it
            blk = t[:, 0:BUF_COLS].rearrange("p (b c) -> p b c", c=E)
            nc.gpsimd.memset(blk[:, :, 0:65], 0.0)
            if nm == "v":
                nc.gpsimd.memset(blk[:, :, 129:130], 0.0)
            else:
                blk4 = t[:, 0:BUF_COLS].rearrange("p (s h c) -> p s h c", h=H, c=E)
                nc.gpsimd.memset(blk4[:, :, 4:8, 66:130], 0.0)

        xt = None
        for m, (tok0, nsub) in enumerate(SCHED):
            if tok0 % INTOK == 0:
                # one packed input DMA covers INTOK tokens of all 4 x parts.
                # SWDGE (gpsimd): an input DMA on a HWDGE ring would
                # head-of-line-block the output stream behind its WAR wait.
                xt = xpool.tile([KC, 4, INTOK], F16, tag="xt", name="xt")
                nc.gpsimd.dma_start(
                    xt[:], xp[:, :, tok0:tok0 + INTOK].rearrange("c p t -> p c t"))
            if m < NSETS:
                for nm in ("q", "k", "v"):
                    _memset_zero_cols(nm, stage[nm][m])
            qs = stage["q"][m % NSETS]
            ks = stage["k"][m % NSETS]
            vs = stage["v"][m % NSETS]
            for s in range(nsub):
                lo = (tok0 % INTOK) + s * SUB
                off = s * HE
                ah = xt[:, 0, lo:lo + SUB]
                al = xt[:, 1, lo:lo + SUB]
                bh = xt[:, 2, lo:lo + SUB]
                bl = xt[:, 3, lo:lo + SUB]
                ps_k = pspool.tile([SUB, 264], F32, tag="ps_k", name="ps_k", bufs=3)
                ps_v = pspool.tile([SUB, 512], F32, tag="ps_v", name="ps_v", bufs=2)
                ps_q = pspool.tile([SUB, 264], F32, tag="ps_q", name="ps_q", bufs=3)
                # x@W = xh@Wh + xh@Wl + xl@Wh (3 accumulating fp16 matmuls)
                for ps, hi, lo_, (w_hi, w_lo) in (
                    (ps_k, ah, al, w_parts["k"]),
                    (ps_v, ah, al, w_parts["v"]),
                    (ps_q, bh, bl, w_parts["q"]),
                ):
                    nc.tensor.matmul(ps[:], hi, w_hi, start=True, stop=False)
                    nc.tensor.matmul(ps[:], hi, w_lo, start=False, stop=False)
                    nc.tensor.matmul(ps[:], lo_, w_hi, start=False, stop=True)

                for ps, st in ((ps_q, qs), (ps_k, ks)):
                    # 65 cols per pair head (the matmul block + its folded
                    # bias col land adjacently).
                    dst = st[:, off + 65:off + 65 + P * E].rearrange(
                        "p (h c) -> p h c", c=E)[:, :, 0:65]
                    src = ps[:, 0:260].rearrange("p (h c) -> p h c", c=65)
                    nc.vector.tensor_copy(dst, src)
                    # single bias col per high head
                    bdst = st[:, off + 585:off + 585 + P * E].rearrange(
                        "p (h c) -> p h c", c=E)[:, :, 0:1]
                    bsrc = ps[:, 260:264].rearrange("p (h c) -> p h c", c=1)
                    nc.vector.tensor_copy(bdst, bsrc)
                vdst = vs[:, off + 65:off + 65 + H * E].rearrange(
                    "p (h c) -> p h c", c=E)[:, :, 0:64]
                vsrc = ps_v[:].rearrange("p (h c) -> p h c", c=64)
                nc.vector.tensor_copy(vdst, vsrc)

            # balance the three output streams across the two HWDGE rings
            ntok = nsub * SUB
            for j, (nm, st) in enumerate((("q", qs), ("k", ks), ("v", vs))):
                eng = nc.sync if (3 * m + j) % 2 == 0 else nc.scalar
                dst = outs[nm][tok0:tok0 + ntok, :].rearrange(
                    "(s p) e -> p s e", p=SUB)
                src = st[:, 0:nsub * HE].rearrange("p (s e) -> p s e", e=HE)
                eng.dma_start(dst, src)
    nc.compile()
    return nc


def _pack_weights(M_q, B_q, M_k, B_k, M_v):
    w = np.zeros((KC, HE), np.float32)
    # K block: cols 0:264.  lhsT rows: 0:64 = x1, 64 = s_mid, 65 = s_last.
    for h in range(P):
        w[0:64, h * 65:h * 65 + 64] = M_k[h].T
        w[65, h * 65 + 64] = B_k[h]          # pair-head bias <- s_last
        w[64, 260 + h] = B_k[P + h]          # high-head bias <- s_mid
    # V block: cols 264:776.
    for h in range(H):
        w[0:64, 264 + h * 64:264 + (h + 1) * 64] = M_v[h].T
    # Q block: cols 776:1040.  lhsT rows: 0:64 = x2, 64 = s_last, 65 = 0.
    for h in range(P):
        w[0:64, 776 + h * 65:776 + h * 65 + 64] = M_q[h].T
        w[64, 776 + h * 65 + 64] = B_q[h]    # pair-head bias <- s_last
        w[64, 1036 + h] = B_q[P + h]         # high-head bias <- s_last
    return w


def _split_f16(a):
    hi = a.astype(np.float16)
    lo = (a - hi.astype(np.float32)).astype(np.float16)
    return hi, lo


def _prep_inputs(inputs):
    x = np.asarray(inputs["x"], np.float32)
    M_q = np.asarray(inputs["M_q"], np.float32)
    B_q = np.asarray(inputs["B_q"], np.float32)[:, 0, 0]
    M_k = np.asarray(inputs["M_k"], np.float32)
    B_k = np.asarray(inputs["B_k"], np.float32)[:, 0, 0]
    M_v = np.asarray(inputs["M_v"], np.float32)
    w = _pack_weights(M_q, B_q, M_k, B_k, M_v)
    w_h, w_l = _split_f16(w)
    wp = np.concatenate([w_h, w_l], axis=1)  # (KC, 2*HE) f16

    in_maps = []
    for b in range(B):
        xt = x[b].T  # (130, 4096) view
        xa = np.empty((KC, N), np.float32)
        xa[0:65] = xt[0:65]        # x1 rows + s_mid row
        xa[65] = xt[129]           # s_last row
        xb = np.empty((KC, N), np.float32)
        xb[0:64] = xt[65:129]      # x2 rows
        xb[64] = xt[129]           # s_last row
        xb[65] = 0.0
        xa_h, xa_l = _split_f16(xa)
        xb_h, xb_l = _split_f16(xb)
        xp = np.stack([xa_h, xa_l, xb_h, xb_l])  # (4, KC, N) f16
        in_maps.append({"xp": xp, "wp": wp})
    return in_maps


def _run(inputs, trace=False):
    if "nc" not in _CACHE:
        _CACHE["nc"] = _build()
    nc = _CACHE["nc"]
    in_maps = _prep_inputs(inputs)
    res = run_bass_kernel_spmd(nc, in_maps, core_ids=list(range(B)), trace=trace)
    q = np.stack([np.asarray(res.results[b]["q"], np.float32) for b in range(B)])
    k = np.stack([np.asarray(res.results[b]["k"], np.float32) for b in range(B)])
    v = np.stack([np.asarray(res.results[b]["v"], np.float32) for b in range(B)])
    return (q, k, v), res


def kernel(**inputs):
    outs, _ = _run(inputs, trace=False)
    return outs



# revision 3
# speedup vs baseline: 2.8551x; 2.8551x over previous
"""Trainium2 Bass kernel for nn_CustomLinear (block-sparse QKV projection).

Given x (8, 4096, 130), per-head 64x64 blocks M_q/M_k (4,64,64), M_v
(8,64,64) and scalar biases B_q/B_k (8,1,1), produces q, k, v each of shape
(8, 4096, 1040) = (B, N, H*E).  Per token row of 1040 floats, only a few
column blocks are nonzero:

  q: head h<4 : cols 130h+65..128  = M_q[h] @ x2,   col 130h+129 = s_last*bq[h]
     head h>=4: col  130h+65       = s_last*bq[h]
  k: head h<4 : cols 130h+65..128  = M_k[h] @ x1,   col 130h+129 = s_last*bk[h]
     head h>=4: col  130h+65       = s_mid*bk[h]
  v: all heads: cols 130h+65..128  = M_v[h] @ x1
  (x1 = x cols 0:64, x2 = x cols 65:129, s_mid = x col 64, s_last = x col 129)

Sharding: pure data parallelism, one batch row per NeuronCore (8 cores),
the tiny weights replicated.

Across q/k/v, exactly 1040 of the 3*1040 output columns per token are ever
nonzero (264 for q, 264 for k, 512 for v); the rest are structural zeros
that depend only on the layout, not the data.  The device therefore
computes a compact (4096, 1040) fp16 tensor per core holding every nonzero
value -- [k 0:264 | v 264:776 | q 776:1040], bias scalars folded into the
matmul contraction -- and the host scatters it into the dense f32 zeros on
unshard.  That cuts device HBM traffic from ~51 MB to ~9.7 MB per core.
fp16 single-pass matmul + fp16 output wire give max rel err ~5e-4 vs the
f32 reference, well under the 2e-2 gate.

Steady state is PE-issue-bound: the PE (pinned at 1.2 GHz on this part;
trace-measured 0.833 ns/moving-col, no HAM ramp over a 31 us stream)
streams 1040 weight columns per 128-token subtile = 867 ns, for a 27.7 us
floor over 32 subtiles.  The PSUM->SBUF casting copies are split DVE: k+q
(~868 ns) / ACT: v (~720 ns) to sit just at that rate, and the two HWDGE
rings carry ~1 MB output DMAs that overlap compute with ~2.4x headroom.

Latency trims around the 27.7 us core: inputs are staged as six small
fully-contiguous DRAM tensors (2-D DMAs, like the weights load -- 3-dim
strided input DMAs on the HWDGE rings hard-fault the device, and SWDGE
costs ~0.8 us Q7 emission each plus queue round-robin that delays the
first chunk by ~4 us) with a tiny 256-token first chunk so the first
matmul can issue ~2.5 us after the framework preamble; the output macro
schedule ramps 1,1,2 subtiles then runs 4s and tapers 2,1,1 so the last
DMA is small and the end-of-kernel receipt wait is short.
"""

import numpy as np
from contextlib import ExitStack

import concourse.bass as bass
import concourse.bacc as bacc
import concourse.mybir as mybir
import concourse.tile as tile
from concourse.bass_utils import run_bass_kernel_spmd

F32 = mybir.dt.float32
F16 = mybir.dt.float16

B = 8            # batches == cores
N = 4096         # tokens per core
D = 64
H = 8            # heads
P = 4            # pair heads
E = 130
KC = 66          # contraction rows: 64 data rows + 2 scalar rows
SUB = 128        # tokens per matmul
NSETS = 4        # stage-buffer sets (output pipeline depth)
# (start, ntok) input chunks: small first chunk so compute starts early.
CHUNKS = [(0, 256), (256, 1792), (2048, 2048)]
# Output DMA macro schedule (tok0, nsub): ramp 1,1,2 / steady 4 / taper 2,1,1.
SCHED = (
    [(0, 1), (SUB, 1), (2 * SUB, 2)]
    + [(t, 4) for t in range(4 * SUB, 28 * SUB, 4 * SUB)]
    + [(28 * SUB, 2), (30 * SUB, 1), (31 * SUB, 1)]
)
assert sum(ns for _, ns in SCHED) == N // SUB

_CACHE = {}


def _build():
    # Bacc (not raw Bass): its compile() legalizes the TRN2 one-sync-wait-
    # per-instruction constraint (move_matmul_waits_to_ldweights +
    # generate_event_semaphores), which walrus codegen hard-requires.
    nc = bacc.Bacc("TRN2", target_bir_lowering=False, debug=False)
    xa_d = [
        nc.dram_tensor(f"xa{c}", [KC, ln], F16, kind="ExternalInput").ap()
        for c, (_, ln) in enumerate(CHUNKS)
    ]
    xb_d = [
        nc.dram_tensor(f"xb{c}", [KC, ln], F16, kind="ExternalInput").ap()
        for c, (_, ln) in enumerate(CHUNKS)
    ]
    wp = nc.dram_tensor("wp", [KC, 1040], F16, kind="ExternalInput").ap()
    out = nc.dram_tensor("out", [N, 1040], F16, kind="ExternalOutput").ap()

    with tile.TileContext(nc) as tc, ExitStack() as ctx:
        wpool = ctx.enter_context(tc.tile_pool(name="wpool", bufs=1))
        xpool = ctx.enter_context(tc.tile_pool(name="xpool", bufs=1))
        opool = ctx.enter_context(tc.tile_pool(name="opool", bufs=1))
        pspool = ctx.enter_context(tc.tile_pool(name="pspool", bufs=2, space="PSUM"))

        # Inputs: fully-contiguous 2-D DMAs on the two HWDGE rings (xa on
        # sync, xb on scalar), smallest chunk first; they have no waits, so
        # they drain before the first output DMA needs the ring.
        xa_t, xb_t = [], []
        wsb = wpool.tile([KC, 1040], F16, name="wsb")
        for c, (_, ln) in enumerate(CHUNKS):
            xa = xpool.tile([KC, ln], F16, name=f"xa{c}")
            nc.sync.dma_start(xa[:], xa_d[c][:])
            if c == 0:
                nc.sync.dma_start(wsb[:], wp[:])
            xb = xpool.tile([KC, ln], F16, name=f"xb{c}")
            nc.scalar.dma_start(xb[:], xb_d[c][:])
            xa_t.append(xa)
            xb_t.append(xb)

        stage = [
            opool.tile([SUB, 4, 1040], F16, name=f"st{i}") for i in range(NSETS)
        ]

        for m, (tok0, nsub) in enumerate(SCHED):
            st = stage[m % NSETS]
            for s in range(nsub):
                tok = tok0 + s * SUB
                c = next(i for i, (t0, ln) in enumerate(CHUNKS)
                         if t0 <= tok < t0 + ln)
                lo = tok - CHUNKS[c][0]
                xa = xa_t[c][:, lo:lo + SUB]
                xb = xb_t[c][:, lo:lo + SUB]
                ps_k = pspool.tile([SUB, 264], F32, tag="ps_k", name="ps_k", bufs=3)
                ps_v = pspool.tile([SUB, 512], F32, tag="ps_v", name="ps_v", bufs=2)
                ps_q = pspool.tile([SUB, 264], F32, tag="ps_q", name="ps_q", bufs=3)
                nc.tensor.matmul(ps_k[:], xa, wsb[:, 0:264], start=True, stop=True)
                nc.tensor.matmul(ps_v[:], xa, wsb[:, 264:776], start=True, stop=True)
                nc.tensor.matmul(ps_q[:], xb, wsb[:, 776:1040], start=True, stop=True)
                # casting f32 PSUM -> f16 stage copies, balanced to the PE's
                # 867 ns/subtile issue rate: DVE (1.04 ns/col + 159 ns/op)
                # takes k+q = 868 ns, ACT (0.833 ns/col + 294 ns/op) takes
                # v = 720 ns.
                nc.vector.tensor_copy(st[:, s, 0:264], ps_k[:])
                nc.vector.tensor_copy(st[:, s, 776:1040], ps_q[:])
                nc.scalar.copy(st[:, s, 264:776], ps_v[:])
            eng = nc.sync if m % 2 == 0 else nc.scalar
            dst = out[tok0:tok0 + nsub * SUB, :].rearrange("(s p) e -> p s e", p=SUB)
            eng.dma_start(dst, st[:, 0:nsub, :])
    nc.compile()
    return nc


def _pack_weights(M_q, B_q, M_k, B_k, M_v):
    # lhsT rows for k/v: 0:64 = x1, 64 = s_mid, 65 = s_last.
    # lhsT rows for q:   0:64 = x2, 64 = s_last, 65 = 0.
    w = np.zeros((KC, 1040), np.float32)
    # K block: cols 0:264 (4 pair heads x 65 [matmul block + bias col], then
    # 4 high-head bias cols).
    for h in range(P):
        w[0:64, h * 65:h * 65 + 64] = M_k[h].T
        w[65, h * 65 + 64] = B_k[h]          # pair-head bias <- s_last
        w[64, 260 + h] = B_k[P + h]          # high-head bias <- s_mid
    # V block: cols 264:776.
    for h in range(H):
        w[0:64, 264 + h * 64:264 + (h + 1) * 64] = M_v[h].T
    # Q block: cols 776:1040.
    for h in range(P):
        w[0:64, 776 + h * 65:776 + h * 65 + 64] = M_q[h].T
        w[64, 776 + h * 65 + 64] = B_q[h]    # pair-head bias <- s_last
        w[64, 1036 + h] = B_q[P + h]         # high-head bias <- s_last
    return w.astype(np.float16)


def _prep_inputs(inputs):
    x = np.asarray(inputs["x"], np.float32)
    M_q = np.asarray(inputs["M_q"], np.float32)
    B_q = np.asarray(inputs["B_q"], np.float32)[:, 0, 0]
    M_k = np.asarray(inputs["M_k"], np.float32)
    B_k = np.asarray(inputs["B_k"], np.float32)[:, 0, 0]
    M_v = np.asarray(inputs["M_v"], np.float32)
    wp = _pack_weights(M_q, B_q, M_k, B_k, M_v)

    in_maps = []
    for b in range(B):
        xt = x[b].T  # (130, 4096) view
        xa = np.empty((KC, N), np.float32)
        xa[0:65] = xt[0:65]        # x1 rows + s_mid row
        xa[65] = xt[129]           # s_last row
        xb = np.zeros((KC, N), np.float32)
        xb[0:64] = xt[65:129]      # x2 rows
        xb[64] = xt[129]           # s_last row
        xa = xa.astype(np.float16)
        xb = xb.astype(np.float16)
        im = {"wp": wp}
        for c, (t0, ln) in enumerate(CHUNKS):
            im[f"xa{c}"] = np.ascontiguousarray(xa[:, t0:t0 + ln])
            im[f"xb{c}"] = np.ascontiguousarray(xb[:, t0:t0 + ln])
        in_maps.append(im)
    return in_maps


def _scatter(o):
    """Scatter the compact (B, N, 1040) f16 device output into dense f32."""
    q = np.zeros((B, N, H * E), np.float32)
    k = np.zeros((B, N, H * E), np.float32)
    v = np.zeros((B, N, H * E), np.float32)
    k_pair = o[:, :, 0:260].reshape(B, N, P, 65)
    q_pair = o[:, :, 776:1036].reshape(B, N, P, 65)
    for h in range(P):
        k[:, :, E * h + 65:E * h + 130] = k_pair[:, :, h]
        q[:, :, E * h + 65:E * h + 130] = q_pair[:, :, h]
        k[:, :, E * (P + h) + 65] = o[:, :, 260 + h]
        q[:, :, E * (P + h) + 65] = o[:, :, 1036 + h]
    vv = o[:, :, 264:776].reshape(B, N, H, 64)
    for h in range(H):
        v[:, :, E * h + 65:E * h + 129] = vv[:, :, h]
    return q, k, v


def _run(inputs, trace=False):
    if "nc" not in _CACHE:
        _CACHE["nc"] = _build()
    nc = _CACHE["nc"]
    in_maps = _prep_inputs(inputs)
    res = run_bass_kernel_spmd(nc, in_maps, core_ids=list(range(B)), trace=trace)
    o = np.stack([np.asarray(res.results[b]["out"]) for b in range(B)])
    return _scatter(o), res


def kernel(**inputs):
    outs, _ = _run(inputs, trace=False)
    return outs


# revision 5
# speedup vs baseline: 3.3906x; 1.1875x over previous
"""Trainium2 Bass kernel for nn_CustomLinear (block-sparse QKV projection).

Given x (8, 4096, 130), per-head 64x64 blocks M_q/M_k (4,64,64), M_v
(8,64,64) and scalar biases B_q/B_k (8,1,1), produces q, k, v each of shape
(8, 4096, 1040) = (B, N, H*E).  Per token row of 1040 floats, only a few
column blocks are nonzero:

  q: head h<4 : cols 130h+65..128  = M_q[h] @ x2,   col 130h+129 = s_last*bq[h]
     head h>=4: col  130h+65       = s_last*bq[h]
  k: head h<4 : cols 130h+65..128  = M_k[h] @ x1,   col 130h+129 = s_last*bk[h]
     head h>=4: col  130h+65       = s_mid*bk[h]
  v: all heads: cols 130h+65..128  = M_v[h] @ x1
  (x1 = x cols 0:64, x2 = x cols 65:129, s_mid = x col 64, s_last = x col 129)

Sharding: pure data parallelism, one batch row per NeuronCore (8 cores),
the tiny weights replicated.

Across q/k/v, exactly 1040 of the 3*1040 output columns per token are ever
nonzero (264 for q, 264 for k, 512 for v); the rest are structural zeros
that depend only on the layout, not the data.  The device therefore
computes a compact (4096, 1040) fp16 tensor per core holding every nonzero
value -- [k 0:264 | q 264:528 | v 528:1040], bias scalars folded into the
matmul contraction -- and the host scatters it into the dense f32 zeros on
unshard.  That cuts device HBM traffic from ~51 MB to ~9.7 MB per core.
fp16 single-pass matmul + fp16 output wire give max rel err ~5e-4 vs the
f32 reference, well under the 2e-2 gate.

Steady state is PE-issue-bound: the PE (pinned at 1.2 GHz on this part;
trace-measured 0.833 ns/moving-col, no HAM ramp over a 31 us stream)
streams 1040 weight columns per 128-token subtile = 867 ns, for a 27.7 us
floor over 32 subtiles.  The PSUM->SBUF casting copies are split DVE: k+q
(~868 ns) / ACT: v (~720 ns) to sit just at that rate, and the two HWDGE
rings carry ~1 MB output DMAs that overlap compute with ~2.4x headroom.

Latency trims around the 27.7 us core: inputs are staged as six small
fully-contiguous DRAM tensors (2-D DMAs, like the weights load -- 3-dim
strided input DMAs on the HWDGE rings hard-fault the device, and SWDGE
costs ~0.8 us Q7 emission each plus queue round-robin that delays the
first chunk by ~4 us) with a tiny 256-token first chunk so the first
matmul can issue ~2.5 us after the framework preamble; the output macro
schedule ramps 1,1,2 subtiles then runs 4s and tapers 2,1,1 so the last
DMA is small and the end-of-kernel receipt wait is short.
"""

import numpy as np
from contextlib import ExitStack

import concourse.bass as bass
import concourse.bacc as bacc
import concourse.mybir as mybir
import concourse.tile as tile
from concourse.bass_utils import run_bass_kernel_spmd

F32 = mybir.dt.float32
F16 = mybir.dt.float16

B = 8            # batches == cores
N = 4096         # tokens per core
D = 64
H = 8            # heads
P = 4            # pair heads
E = 130
KC = 66          # contraction rows: 64 data rows + 2 scalar rows
SUB = 128        # tokens per matmul
NSETS = 4        # stage-buffer sets (output pipeline depth)
# (start, ntok) input chunks.  HBM->SBUF loads only sustain ~90-140 GB/s
# (66 latency-bound descriptors per DMA), so chunks grow geometrically:
# each chunk's ~2 us completion latency hides behind the ~867 ns/subtile
# compute of the previous one.
CHUNKS = [(0, 512), (512, 1024), (1536, 2560)]
# Output DMA macro schedule (tok0, nsub): ramp 1,1,2 / steady 4 / taper 2,1,1.
SCHED = (
    [(0, 1), (SUB, 1), (2 * SUB, 2)]
    + [(t, 4) for t in range(4 * SUB, 28 * SUB, 4 * SUB)]
    + [(28 * SUB, 2), (30 * SUB, 1), (31 * SUB, 1)]
)
assert sum(ns for _, ns in SCHED) == N // SUB

_CACHE = {}


def _build():
    # Bacc (not raw Bass): its compile() legalizes the TRN2 one-sync-wait-
    # per-instruction constraint (move_matmul_waits_to_ldweights +
    # generate_event_semaphores), which walrus codegen hard-requires.
    nc = bacc.Bacc("TRN2", target_bir_lowering=False, debug=False)
    xa_d = [
        nc.dram_tensor(f"xa{c}", [KC, ln], F16, kind="ExternalInput").ap()
        for c, (_, ln) in enumerate(CHUNKS)
    ]
    xb_d = [
        nc.dram_tensor(f"xb{c}", [KC, ln], F16, kind="ExternalInput").ap()
        for c, (_, ln) in enumerate(CHUNKS)
    ]
    wp = nc.dram_tensor("wp", [KC, 1040], F16, kind="ExternalInput").ap()
    out = nc.dram_tensor("out", [N, 1040], F16, kind="ExternalOutput").ap()

    with tile.TileContext(nc) as tc, ExitStack() as ctx:
        wpool = ctx.enter_context(tc.tile_pool(name="wpool", bufs=1))
        xpool = ctx.enter_context(tc.tile_pool(name="xpool", bufs=1))
        opool = ctx.enter_context(tc.tile_pool(name="opool", bufs=1))
        pspool = ctx.enter_context(tc.tile_pool(name="pspool", bufs=2, space="PSUM"))

        # Inputs: fully-contiguous 2-D DMAs on the two HWDGE rings (xa on
        # sync, xb on scalar), smallest chunk first; they have no waits, so
        # they drain before the first output DMA needs the ring.
        xa_t, xb_t = [], []
        wsb = wpool.tile([KC, 1040], F16, name="wsb")
        for c, (_, ln) in enumerate(CHUNKS):
            xa = xpool.tile([KC, ln], F16, name=f"xa{c}")
            nc.sync.dma_start(xa[:], xa_d[c][:])
            if c == 0:
                nc.sync.dma_start(wsb[:], wp[:])
            xb = xpool.tile([KC, ln], F16, name=f"xb{c}")
            nc.scalar.dma_start(xb[:], xb_d[c][:])
            xa_t.append(xa)
            xb_t.append(xb)

        stage = [
            opool.tile([SUB, 4, 1040], F16, name=f"st{i}") for i in range(NSETS)
        ]

        for m, (tok0, nsub) in enumerate(SCHED):
            st = stage[m % NSETS]
            for s in range(nsub):
                tok = tok0 + s * SUB
                c = next(i for i, (t0, ln) in enumerate(CHUNKS)
                         if t0 <= tok < t0 + ln)
                lo = tok - CHUNKS[c][0]
                xa = xa_t[c][:, lo:lo + SUB]
                xb = xb_t[c][:, lo:lo + SUB]
                # k and q land in the two banks of one PSUM tile so a
                # single strided-AP DVE cast moves both (per-op overhead on
                # the DVE is ~350 ns; two separate casts exceed the PE rate).
                ps_kq = pspool.tile([SUB, 2, 512], F32, tag="ps_kq", name="ps_kq", bufs=3)
                ps_v = pspool.tile([SUB, 512], F32, tag="ps_v", name="ps_v", bufs=2)
                nc.tensor.matmul(ps_kq[:, 0, 0:264], xa, wsb[:, 0:264], start=True, stop=True)
                nc.tensor.matmul(ps_v[:], xa, wsb[:, 264:776], start=True, stop=True)
                nc.tensor.matmul(ps_kq[:, 1, 0:264], xb, wsb[:, 776:1040], start=True, stop=True)
                # casting f32 PSUM -> f16 stage copies, balanced to the PE's
                # 867 ns/subtile issue rate: DVE takes k+q (~790 ns), ACT
                # takes v (~824 ns).
                dst_kq = st[:, s, 0:528].rearrange("p (g c) -> p g c", c=264)
                nc.vector.tensor_copy(dst_kq, ps_kq[:, :, 0:264])
                nc.scalar.copy(st[:, s, 528:1040], ps_v[:])
            # output DMA triggers stay off the ACT engine (it is ~95% busy
            # with v casts): alternate gpsimd SWDGE with the sync HWDGE ring,
            # odd parity so the final taper macro rides the lower-latency
            # HWDGE ring.
            eng = nc.gpsimd if m % 2 == 0 else nc.sync
            dst = out[tok0:tok0 + nsub * SUB, :].rearrange("(s p) e -> p s e", p=SUB)
            eng.dma_start(dst, st[:, 0:nsub, :])
    nc.compile()
    return nc


def _pack_weights(M_q, B_q, M_k, B_k, M_v):
    # lhsT rows for k/v: 0:64 = x1, 64 = s_mid, 65 = s_last.
    # lhsT rows for q:   0:64 = x2, 64 = s_last, 65 = 0.
    w = np.zeros((KC, 1040), np.float32)
    # K block: cols 0:264 (4 pair heads x 65 [matmul block + bias col], then
    # 4 high-head bias cols).
    for h in range(P):
        w[0:64, h * 65:h * 65 + 64] = M_k[h].T
        w[65, h * 65 + 64] = B_k[h]          # pair-head bias <- s_last
        w[64, 260 + h] = B_k[P + h]          # high-head bias <- s_mid
    # V block: cols 264:776.
    for h in range(H):
        w[0:64, 264 + h * 64:264 + (h + 1) * 64] = M_v[h].T
    # Q block: cols 776:1040.
    for h in range(P):
        w[0:64, 776 + h * 65:776 + h * 65 + 64] = M_q[h].T
        w[64, 776 + h * 65 + 64] = B_q[h]    # pair-head bias <- s_last
        w[64, 1036 + h] = B_q[P + h]         # high-head bias <- s_last
    return w.astype(np.float16)


def _prep_inputs(inputs):
    x = np.asarray(inputs["x"], np.float32)
    M_q = np.asarray(inputs["M_q"], np.float32)
    B_q = np.asarray(inputs["B_q"], np.float32)[:, 0, 0]
    M_k = np.asarray(inputs["M_k"], np.float32)
    B_k = np.asarray(inputs["B_k"], np.float32)[:, 0, 0]
    M_v = np.asarray(inputs["M_v"], np.float32)
    wp = _pack_weights(M_q, B_q, M_k, B_k, M_v)

    in_maps = []
    for b in range(B):
        xt = x[b].T  # (130, 4096) view
        xa = np.empty((KC, N), np.float32)
        xa[0:65] = xt[0:65]        # x1 rows + s_mid row
        xa[65] = xt[129]           # s_last row
        xb = np.zeros((KC, N), np.float32)
        xb[0:64] = xt[65:129]      # x2 rows
        xb[64] = xt[129]           # s_last row
        xa = xa.astype(np.float16)
        xb = xb.astype(np.float16)
        im = {"wp": wp}
        for c, (t0, ln) in enumerate(CHUNKS):
            im[f"xa{c}"] = np.ascontiguousarray(xa[:, t0:t0 + ln])
            im[f"xb{c}"] = np.ascontiguousarray(xb[:, t0:t0 + ln])
        in_maps.append(im)
    return in_maps


def _scatter(o):
    """Scatter the compact (B, N, 1040) f16 device output into dense f32."""
    q = np.zeros((B, N, H * E), np.float32)
    k = np.zeros((B, N, H * E), np.float32)
    v = np.zeros((B, N, H * E), np.float32)
    k_pair = o[:, :, 0:260].reshape(B, N, P, 65)
    q_pair = o[:, :, 264:524].reshape(B, N, P, 65)
    for h in range(P):
        k[:, :, E * h + 65:E * h + 130] = k_pair[:, :, h]
        q[:, :, E * h + 65:E * h + 130] = q_pair[:, :, h]
        k[:, :, E * (P + h) + 65] = o[:, :, 260 + h]
        q[:, :, E * (P + h) + 65] = o[:, :, 524 + h]
    vv = o[:, :, 528:1040].reshape(B, N, H, 64)
    for h in range(H):
        v[:, :, E * h + 65:E * h + 129] = vv[:, :, h]
    return q, k, v


def _run(inputs, trace=False):
    if "nc" not in _CACHE:
        _CACHE["nc"] = _build()
    nc = _CACHE["nc"]
    in_maps = _prep_inputs(inputs)
    res = run_bass_kernel_spmd(nc, in_maps, core_ids=list(range(B)), trace=trace)
    o = np.stack([np.asarray(res.results[b]["out"]) for b in range(B)])
    return _scatter(o), res


def kernel(**inputs):
    outs, _ = _run(inputs, trace=False)
    return outs


# revision 6
# speedup vs baseline: 3.4700x; 1.0234x over previous
"""Trainium2 Bass kernel for nn_CustomLinear (block-sparse QKV projection).

Given x (8, 4096, 130), per-head 64x64 blocks M_q/M_k (4,64,64), M_v
(8,64,64) and scalar biases B_q/B_k (8,1,1), produces q, k, v each of shape
(8, 4096, 1040) = (B, N, H*E).  Per token row of 1040 floats, only a few
column blocks are nonzero:

  q: head h<4 : cols 130h+65..128  = M_q[h] @ x2,   col 130h+129 = s_last*bq[h]
     head h>=4: col  130h+65       = s_last*bq[h]
  k: head h<4 : cols 130h+65..128  = M_k[h] @ x1,   col 130h+129 = s_last*bk[h]
     head h>=4: col  130h+65       = s_mid*bk[h]
  v: all heads: cols 130h+65..128  = M_v[h] @ x1
  (x1 = x cols 0:64, x2 = x cols 65:129, s_mid = x col 64, s_last = x col 129)

Sharding: pure data parallelism, one batch row per NeuronCore (8 cores),
the tiny weights replicated.

Across q/k/v, exactly 1040 of the 3*1040 output columns per token are ever
nonzero (264 for q, 264 for k, 512 for v); the rest are structural zeros
that depend only on the layout, not the data.  The device therefore
computes a compact (4096, 1040) fp16 tensor per core holding every nonzero
value -- [k 0:264 | q 264:528 | v 528:1040], bias scalars folded into the
matmul contraction -- and the host scatters it into the dense f32 zeros on
unshard.  That cuts device HBM traffic from ~51 MB to ~9.7 MB per core.
fp16 single-pass matmul + fp16 output wire give max rel err ~5e-4 vs the
f32 reference, well under the 2e-2 gate.

Steady state is PE-issue-bound: the PE (pinned at 1.2 GHz on this part;
trace-measured 0.833 ns/moving-col, no HAM ramp over a 31 us stream)
streams 1040 weight columns per 128-token subtile = 867 ns, for a 27.7 us
floor over 32 subtiles.  The PSUM->SBUF casting copies are split DVE: k+q
(~868 ns) / ACT: v (~720 ns) to sit just at that rate, and the two HWDGE
rings carry ~1 MB output DMAs that overlap compute with ~2.4x headroom.

Latency trims around the 27.7 us core: inputs are staged as six small
fully-contiguous DRAM tensors (2-D DMAs, like the weights load -- 3-dim
strided input DMAs on the HWDGE rings hard-fault the device, and SWDGE
costs ~0.8 us Q7 emission each plus queue round-robin that delays the
first chunk by ~4 us) with a tiny 256-token first chunk so the first
matmul can issue ~2.5 us after the framework preamble; the output macro
schedule ramps 1,1,2 subtiles then runs 4s and tapers 2,1,1 so the last
DMA is small and the end-of-kernel receipt wait is short.
"""

import numpy as np
from contextlib import ExitStack

import concourse.bass as bass
import concourse.bacc as bacc
import concourse.mybir as mybir
import concourse.tile as tile
from concourse.bass_utils import run_bass_kernel_spmd

F32 = mybir.dt.float32
F16 = mybir.dt.float16

B = 8            # batches == cores
N = 4096         # tokens per core
D = 64
H = 8            # heads
P = 4            # pair heads
E = 130
KC = 66          # contraction rows: 64 data rows + 2 scalar rows
SUB = 128        # tokens per matmul
NSETS = 4        # stage-buffer sets (output pipeline depth)
# (start, ntok) input chunks.  HBM->SBUF loads only sustain ~90-140 GB/s
# (66 latency-bound descriptors per DMA), so chunks grow geometrically:
# each chunk's ~2 us completion latency hides behind the ~867 ns/subtile
# compute of the previous one.
CHUNKS = [(0, 512), (512, 1024), (1536, 2560)]
# Output DMA macro schedule (tok0, nsub): ramp 1,1,2 / steady 4 / taper 2,1,1.
SCHED = (
    [(0, 1), (SUB, 1), (2 * SUB, 2)]
    + [(t, 4) for t in range(4 * SUB, 28 * SUB, 4 * SUB)]
    + [(28 * SUB, 2), (30 * SUB, 1), (31 * SUB, 1)]
)
assert sum(ns for _, ns in SCHED) == N // SUB

_CACHE = {}


def _build():
    # Bacc (not raw Bass): its compile() legalizes the TRN2 one-sync-wait-
    # per-instruction constraint (move_matmul_waits_to_ldweights +
    # generate_event_semaphores), which walrus codegen hard-requires.
    nc = bacc.Bacc("TRN2", target_bir_lowering=False, debug=False)
    xa_d = [
        nc.dram_tensor(f"xa{c}", [KC, ln], F16, kind="ExternalInput").ap()
        for c, (_, ln) in enumerate(CHUNKS)
    ]
    xb_d = [
        nc.dram_tensor(f"xb{c}", [KC, ln], F16, kind="ExternalInput").ap()
        for c, (_, ln) in enumerate(CHUNKS)
    ]
    wp = nc.dram_tensor("wp", [KC, 1040], F16, kind="ExternalInput").ap()
    out = nc.dram_tensor("out", [N, 1040], F16, kind="ExternalOutput").ap()

    with tile.TileContext(nc) as tc, ExitStack() as ctx:
        wpool = ctx.enter_context(tc.tile_pool(name="wpool", bufs=1))
        xpool = ctx.enter_context(tc.tile_pool(name="xpool", bufs=1))
        opool = ctx.enter_context(tc.tile_pool(name="opool", bufs=1))
        pspool = ctx.enter_context(tc.tile_pool(name="pspool", bufs=2, space="PSUM"))

        # Inputs: fully-contiguous 2-D DMAs on the two HWDGE rings (xa on
        # sync, xb on scalar), smallest chunk first; they have no waits, so
        # they drain before the first output DMA needs the ring.
        xa_t, xb_t = [], []
        wsb = wpool.tile([KC, 1040], F16, name="wsb")
        for c, (_, ln) in enumerate(CHUNKS):
            xa = xpool.tile([KC, ln], F16, name=f"xa{c}")
            nc.sync.dma_start(xa[:], xa_d[c][:])
            if c == 0:
                nc.sync.dma_start(wsb[:], wp[:])
            xb = xpool.tile([KC, ln], F16, name=f"xb{c}")
            nc.scalar.dma_start(xb[:], xb_d[c][:])
            xa_t.append(xa)
            xb_t.append(xb)

        stage = [
            opool.tile([SUB, 4, 1040], F16, name=f"st{i}") for i in range(NSETS)
        ]

        for m, (tok0, nsub) in enumerate(SCHED):
            st = stage[m % NSETS]
            for s in range(nsub):
                tok = tok0 + s * SUB
                c = next(i for i, (t0, ln) in enumerate(CHUNKS)
                         if t0 <= tok < t0 + ln)
                lo = tok - CHUNKS[c][0]
                xa = xa_t[c][:, lo:lo + SUB]
                xb = xb_t[c][:, lo:lo + SUB]
                # k and q land in the two banks of one PSUM tile so a
                # single strided-AP DVE cast moves both (per-op overhead on
                # the DVE is ~350 ns; two separate casts exceed the PE rate).
                ps_kq = pspool.tile([SUB, 2, 512], F32, tag="ps_kq", name="ps_kq", bufs=3)
                ps_v = pspool.tile([SUB, 512], F32, tag="ps_v", name="ps_v", bufs=2)
                nc.tensor.matmul(ps_kq[:, 0, 0:264], xa, wsb[:, 0:264], start=True, stop=True)
                nc.tensor.matmul(ps_v[:], xa, wsb[:, 264:776], start=True, stop=True)
                nc.tensor.matmul(ps_kq[:, 1, 0:264], xb, wsb[:, 776:1040], start=True, stop=True)
                # casting f32 PSUM -> f16 stage copies, balanced to the PE's
                # 867 ns/subtile issue rate: DVE takes k+q (~790 ns), ACT
                # takes v (~824 ns).
                dst_kq = st[:, s, 0:528].rearrange("p (g c) -> p g c", c=264)
                nc.vector.tensor_copy(dst_kq, ps_kq[:, :, 0:264])
                nc.scalar.copy(st[:, s, 528:1040], ps_v[:])
            # output DMA triggers stay off the ACT engine (it is ~95% busy
            # with v casts) while compute runs: alternate gpsimd SWDGE with
            # the sync HWDGE ring.  The last three taper macros each get
            # their own queue (ACT is idle by then) so their ~2 us HBM-write
            # completion receipts overlap instead of serializing in one
            # ring's FIFO.
            if m == len(SCHED) - 1:
                eng = nc.scalar
            elif m == len(SCHED) - 2:
                eng = nc.sync
            elif m == len(SCHED) - 3:
                eng = nc.gpsimd
            else:
                eng = nc.gpsimd if m % 2 == 0 else nc.sync
            dst = out[tok0:tok0 + nsub * SUB, :].rearrange("(s p) e -> p s e", p=SUB)
            eng.dma_start(dst, st[:, 0:nsub, :])
    nc.compile()
    return nc


def _pack_weights(M_q, B_q, M_k, B_k, M_v):
    # lhsT rows for k/v: 0:64 = x1, 64 = s_mid, 65 = s_last.
    # lhsT rows for q:   0:64 = x2, 64 = s_last, 65 = 0.
    w = np.zeros((KC, 1040), np.float32)
    # K block: cols 0:264 (4 pair heads x 65 [matmul block + bias col], then
    # 4 high-head bias cols).
    for h in range(P):
        w[0:64, h * 65:h * 65 + 64] = M_k[h].T
        w[65, h * 65 + 64] = B_k[h]          # pair-head bias <- s_last
        w[64, 260 + h] = B_k[P + h]          # high-head bias <- s_mid
    # V block: cols 264:776.
    for h in range(H):
        w[0:64, 264 + h * 64:264 + (h + 1) * 64] = M_v[h].T
    # Q block: cols 776:1040.
    for h in range(P):
        w[0:64, 776 + h * 65:776 + h * 65 + 64] = M_q[h].T
        w[64, 776 + h * 65 + 64] = B_q[h]    # pair-head bias <- s_last
        w[64, 1036 + h] = B_q[P + h]         # high-head bias <- s_last
    return w.astype(np.float16)


def _prep_inputs(inputs):
    x = np.asarray(inputs["x"], np.float32)
    M_q = np.asarray(inputs["M_q"], np.float32)
    B_q = np.asarray(inputs["B_q"], np.float32)[:, 0, 0]
    M_k = np.asarray(inputs["M_k"], np.float32)
    B_k = np.asarray(inputs["B_k"], np.float32)[:, 0, 0]
    M_v = np.asarray(inputs["M_v"], np.float32)
    wp = _pack_weights(M_q, B_q, M_k, B_k, M_v)

    in_maps = []
    for b in range(B):
        xt = x[b].T  # (130, 4096) view
        xa = np.empty((KC, N), np.float32)
        xa[0:65] = xt[0:65]        # x1 rows + s_mid row
        xa[65] = xt[129]           # s_last row
        xb = np.zeros((KC, N), np.float32)
        xb[0:64] = xt[65:129]      # x2 rows
        xb[64] = xt[129]           # s_last row
        xa = xa.astype(np.float16)
        xb = xb.astype(np.float16)
        im = {"wp": wp}
        for c, (t0, ln) in enumerate(CHUNKS):
            im[f"xa{c}"] = np.ascontiguousarray(xa[:, t0:t0 + ln])
            im[f"xb{c}"] = np.ascontiguousarray(xb[:, t0:t0 + ln])
        in_maps.append(im)
    return in_maps


def _scatter(o):
    """Scatter the compact (B, N, 1040) f16 device output into dense f32."""
    q = np.zeros((B, N, H * E), np.float32)
    k = np.zeros((B, N, H * E), np.float32)
    v = np.zeros((B, N, H * E), np.float32)
    k_pair = o[:, :, 0:260].reshape(B, N, P, 65)
    q_pair = o[:, :, 264:524].reshape(B, N, P, 65)
    for h in range(P):
        k[:, :, E * h + 65:E * h + 130] = k_pair[:, :, h]
        q[:, :, E * h + 65:E * h + 130] = q_pair[:, :, h]
        k[:, :, E * (P + h) + 65] = o[:, :, 260 + h]
        q[:, :, E * (P + h) + 65] = o[:, :, 524 + h]
    vv = o[:, :, 528:1040].reshape(B, N, H, 64)
    for h in range(H):
        v[:, :, E * h + 65:E * h + 129] = vv[:, :, h]
    return q, k, v


def _run(inputs, trace=False):
    if "nc" not in _CACHE:
        _CACHE["nc"] = _build()
    nc = _CACHE["nc"]
    in_maps = _prep_inputs(inputs)
    res = run_bass_kernel_spmd(nc, in_maps, core_ids=list(range(B)), trace=trace)
    o = np.stack([np.asarray(res.results[b]["out"]) for b in range(B)])
    return _scatter(o), res


def kernel(**inputs):
    outs, _ = _run(inputs, trace=False)
    return outs
